# revision 1
# baseline (speedup 1.0000x reference)
"""Trainium2 Bass kernel for nn_ARDecoderECD (GRU->LSTM AR decoder).

Strategy (pure data-parallel over 8 NeuronCores, batch-sharded):
  - layout: hidden dim on SBUF partitions, batch on the free dim
  - embedding + GRU input projection folded into a 23-row table applied via
    one-hot matmul (one-hot computed on host, streamed from DRAM)
  - all GRU biases folded into the table / a one-hot bias matmul
  - LSTM + output biases folded into an augmented ones-row of the LSTM state
  - all matmuls in float32r (full PE speed at N=512, fp32 storage)
  - elementwise split across ACT / DVE / GPSIMD
"""

import numpy as np
from contextlib import ExitStack

import concourse.bacc as bacc
import concourse.bass as bass
import concourse.tile as tile
from concourse import mybir
from concourse.bass_utils import run_bass_kernel_spmd
from concourse import hw_specs as _hw_specs

# Calibrate the Tile scheduler's cost model to measured TRN2 per-op fixed
# overheads (ScalarE ~(N+352)cyc, VectorE ~(N+151)cyc SBUF). This only
# changes instruction-ordering decisions and TimelineSim predictions.
import os as _os
if _os.environ.get("TRN_CALIB", "0") == "1":
    _hw_specs.TRN2Spec.ACCESS_CYCLES = {
        **_hw_specs.TRN2Spec.ACCESS_CYCLES,
        (bass.MemorySpace.SBUF, mybir.EngineType.DVE): 151,
        (bass.MemorySpace.PSUM, mybir.EngineType.DVE): 213,
        (bass.MemorySpace.SBUF, mybir.EngineType.Activation): 352,
        (bass.MemorySpace.PSUM, mybir.EngineType.Activation): 322,
    }

B, T = 8192, 26
V, E, H, L = 23, 100, 128, 64
N_CORES = 8
BC = B // N_CORES  # 1024 samples per core
F32 = mybir.dt.float32
F32R = mybir.dt.float32r
AF = mybir.ActivationFunctionType
ALU = mybir.AluOpType
PRIO_OFF = 0
N_CHAINS = 4
TC_PAIR = False


def _emit(nc, tc, d, ctx, reps=1):
    """Emit the per-core kernel. d maps names -> DRAM APs.

    Two independent batch chains of 512 are interleaved with a half-step
    stagger (chain1's GRU runs alongside chain0's LSTM) so the serial
    recurrence backbone of one chain overlaps bulk engine work of the other.
    The output projection copy/DMA of step t is deferred to step t+1.
    """
    wp = ctx.enter_context(tc.tile_pool(name="weights", bufs=1))
    run = ctx.enter_context(tc.tile_pool(name="run", bufs=2))
    opool = ctx.enter_context(tc.tile_pool(name="opool", bufs=3))
    pp = ctx.enter_context(tc.tile_pool(name="psum", bufs=1, space="PSUM"))

    def mm(out, lhsT, rhs, start, stop):
        nc.tensor.matmul(out, lhsT, rhs, start=start, stop=stop)

    # ---- load weights ----
    def wload(name, shape, dt_=F32R):
        t = wp.tile(shape, dt_, name=name)
        nc.sync.dma_start(t[:], d[name][:])
        return t

    xg_sb = wload("xg_tab", [V, 3 * H])
    whh_sb = wload("whh_T", [H, 3 * H])
    fcw_sb = wload("fcw_T", [L, H])
    fcb_sb = wload("fc_b", [H, 1], F32)
    bhhn_sb = wload("bhh_n", [H, 1], F32)
    wihl_sb = wload("wih_lT", [H, 4 * E])
    whhl_sb = wload("whh_laug", [E + 1, 4 * E])
    outw_sb = wload("out_waug", [E + 1, V])
    zT_sb = wload("zT", [L, BC])

    NCH = N_CHAINS     # independent batch chains per core
    CW = BC // NCH     # chain width (512)
    chs = [slice(c * CW, (c + 1) * CW) for c in range(NCH)]

    # LSTM state ping-pong tiles (per chain) with persistent ones-row (row E)
    hl_t = [[None, None] for _ in range(NCH)]
    for c in range(NCH):
        for idx in range(2):
            hlx = wp.tile([E + 1, CW], F32R, name=f"hl{c}_{idx}")
            nc.sync.dma_start(hlx[:], d["hl_init"][:, 0:CW])
            hl_t[c][idx] = hlx

    for rep in range(reps):
        h = [None] * NCH
        c_prev = [None] * NCH
        pend_out = [None] * NCH   # deferred output projection (pout, t)
        c_pair = [None] * NCH
        O_tiles = {}

        def load_O(t):
            Ot = opool.tile([V, BC], F32R, tag="O", name=f"O{t}_{rep}")
            nc.sync.dma_start(Ot[:], d["O"][t])
            O_tiles[t] = Ot

        def flush_out(c):
            if pend_out[c] is None:
                return
            pout, t_ = pend_out[c]
            out_sb = run.tile([V, CW], F32, tag=f"out{c}",
                              name=f"out{t_}_{c}", uniquify=True)
            nc.vector.tensor_copy(out_sb[:], pout[:])
            nc.sync.dma_start(d["logits"][t_][:, chs[c]], out_sb[:])
            pend_out[c] = None

        def emit_h0(c):
            ph0 = pp.tile([H, CW], F32, tag=f"pA{c}", name=f"ph0_{rep}_{c}")
            mm(ph0[:], fcw_sb[:], zT_sb[:, chs[c]], start=True, stop=True)
            hc = run.tile([H, CW], F32R, tag=f"h{c}", name=f"h_init_{rep}_{c}")
            nc.scalar.activation(hc[:], ph0[:], AF.Tanh, bias=fcb_sb[:, 0:1])
            h[c] = hc

        def emit_gru(t, c):
            flush_out(c)
            Ot = O_tiles[t]
            prz = pp.tile([H, 2 * CW], F32, tag=f"pA{c}",
                          name=f"prz{t}_{c}_{rep}")
            pxh = pp.tile([H, 2 * CW], F32, tag=f"pB{c}",
                          name=f"pxh{t}_{c}_{rep}")
            mm(prz[:, 0:CW], xg_sb[:, 0:H], Ot[:, chs[c]],
               start=True, stop=False)
            mm(prz[:, 0:CW], whh_sb[:, 0:H], h[c][:], start=False, stop=True)
            mm(prz[:, CW:], xg_sb[:, H : 2 * H], Ot[:, chs[c]],
               start=True, stop=False)
            mm(prz[:, CW:], whh_sb[:, H : 2 * H], h[c][:],
               start=False, stop=True)
            mm(pxh[:, 0:CW], xg_sb[:, 2 * H : 3 * H], Ot[:, chs[c]],
               start=True, stop=True)
            mm(pxh[:, CW:], whh_sb[:, 2 * H : 3 * H], h[c][:],
               start=True, stop=True)

            rz_sb = run.tile([H, 2 * CW], F32, tag=f"rz{c}",
                             name=f"rz{t}_{c}_{rep}")
            nc.scalar.activation(rz_sb[:], prz[:], AF.Sigmoid)
            # off-backbone pieces
            oz_sb = run.tile([H, CW], F32, tag=f"oz{c}",
                             name=f"oz{t}_{c}_{rep}")
            nc.vector.tensor_scalar(oz_sb[:], rz_sb[:, CW:], -1.0, 1.0,
                                    ALU.mult, ALU.add)
            zh_sb = run.tile([H, CW], F32, tag=f"zh{c}",
                             name=f"zh{t}_{c}_{rep}")
            nc.gpsimd.tensor_mul(zh_sb[:], rz_sb[:, CW:], h[c][:].bitcast(F32))
            # backbone: n-gate then blend
            t1_sb = run.tile([H, CW], F32, tag=f"t1{c}", name=f"t1{t}_{c}_{rep}")
            nc.vector.scalar_tensor_tensor(
                t1_sb[:], pxh[:, CW:], bhhn_sb[:, 0:1], rz_sb[:, 0:CW],
                ALU.add, ALU.mult)
            t2_sb = run.tile([H, CW], F32, tag=f"t2{c}", name=f"t2{t}_{c}_{rep}")
            nc.vector.tensor_add(t2_sb[:], t1_sb[:], pxh[:, 0:CW])
            n_sb = run.tile([H, CW], F32, tag=f"n{c}", name=f"n{t}_{c}_{rep}")
            nc.scalar.activation(n_sb[:], t2_sb[:], AF.Tanh)
            nz_sb = run.tile([H, CW], F32, tag=f"nz{c}", name=f"nz{t}_{c}_{rep}")
            nc.vector.tensor_mul(nz_sb[:], n_sb[:], oz_sb[:])
            h_new = run.tile([H, CW], F32R, tag=f"h{c}", name=f"h{t}_{c}_{rep}")
            nc.vector.tensor_add(h_new[:], nz_sb[:], zh_sb[:])
            h[c] = h_new

        o_gate = [None] * NCH
        tc_pairs = [None] * NCH

        def emit_lstm_front(t, c):
            hl_prev = hl_t[c][t % 2]
            pif = pp.tile([E, 2 * CW], F32, tag=f"pA{c}",
                          name=f"pif{t}_{c}_{rep}")
            pgo = pp.tile([E, 2 * CW], F32, tag=f"pB{c}",
                          name=f"pgo{t}_{c}_{rep}")
            for gi, (ps, reg) in enumerate(
                    [(pif, slice(0, CW)), (pif, slice(CW, 2 * CW)),
                     (pgo, slice(0, CW)), (pgo, slice(CW, 2 * CW))]):
                gs = slice(gi * E, (gi + 1) * E)
                mm(ps[:, reg], wihl_sb[:, gs], h[c][:], start=True, stop=False)
                mm(ps[:, reg], whhl_sb[:, gs], hl_prev[:],
                   start=False, stop=True)

            if_sb = run.tile([E, 2 * CW], F32, tag=f"if{c}",
                             name=f"if{t}_{c}_{rep}")
            nc.scalar.activation(if_sb[:], pif[:], AF.Sigmoid)
            g_sb = run.tile([E, CW], F32, tag=f"gg{c}", name=f"g{t}_{c}_{rep}")
            nc.scalar.activation(g_sb[:], pgo[:, 0:CW], AF.Tanh)
            o_sb = run.tile([E, CW], F32, tag=f"og{c}", name=f"o{t}_{c}_{rep}")
            nc.scalar.activation(o_sb[:], pgo[:, CW:], AF.Sigmoid)
            o_gate[c] = o_sb

            if TC_PAIR:
                p, side = c // 2, c % 2
                half = slice(side * CW, (side + 1) * CW)
                if side == 0:
                    c_pair[p] = run.tile([E, 2 * CW], F32, tag=f"cp{p}",
                                         name=f"cp{t}_{p}_{rep}")
            else:
                p, half = c, slice(0, CW)
                c_pair[p] = run.tile([E, CW], F32, tag=f"cp{p}",
                                     name=f"cp{t}_{p}_{rep}")
            cp = c_pair[p]
            if t == 0:
                nc.vector.tensor_mul(cp[:, half], if_sb[:, 0:CW], g_sb[:])
            else:
                m1_sb = run.tile([E, CW], F32, tag=f"m1{c}",
                                 name=f"m1{t}_{c}_{rep}")
                nc.gpsimd.tensor_mul(m1_sb[:], if_sb[:, CW:],
                                     c_prev[c][:])
                m2_sb = run.tile([E, CW], F32, tag=f"m2{c}",
                                 name=f"m2{t}_{c}_{rep}")
                nc.vector.tensor_mul(m2_sb[:], if_sb[:, 0:CW], g_sb[:])
                nc.vector.tensor_add(cp[:, half], m1_sb[:], m2_sb[:])
            c_prev[c] = cp[:, half]

        def emit_tc_pair(t, p):
            w = 2 * CW if TC_PAIR else CW
            tcp = run.tile([E, w], F32, tag=f"tcp{p}",
                           name=f"tcp{t}_{p}_{rep}")
            nc.scalar.activation(tcp[:], c_pair[p][:], AF.Tanh)
            tc_pairs[p] = tcp

        def emit_lstm_back(t, c):
            hl_new = hl_t[c][(t + 1) % 2]
            if TC_PAIR:
                p = c // 2
                half = slice((c % 2) * CW, (c % 2 + 1) * CW)
            else:
                p, half = c, slice(0, CW)
            nc.gpsimd.tensor_mul(hl_new[0:E, :], o_gate[c][:],
                                 tc_pairs[p][:, half])
            pout = pp.tile([V, CW], F32, tag=f"pB{c}", name=f"pout{t}_{c}_{rep}")
            mm(pout[:], outw_sb[:], hl_new[:], start=True, stop=True)
            pend_out[c] = (pout, t)

        for c in range(NCH):
            emit_h0(c)
        load_O(0)
        load_O(1)
        for t in range(T):
            if t + 2 < T:
                load_O(t + 2)
            for c in range(NCH):
                emit_gru(t, c)
            if TC_PAIR:
                for p in range(NCH // 2):
                    emit_lstm_front(t, 2 * p)
                    emit_lstm_front(t, 2 * p + 1)
                    emit_tc_pair(t, p)
            else:
                for c in range(NCH):
                    emit_lstm_front(t, c)
                    emit_tc_pair(t, c)
            for c in range(NCH):
                emit_lstm_back(t, c)
        for c in range(NCH):
            flush_out(c)


def _host_prep(inputs):
    f32 = np.float32
    emb = np.asarray(inputs["emb"], f32)
    gru_wih = np.asarray(inputs["gru_wih"], f32)
    gru_whh = np.asarray(inputs["gru_whh"], f32)
    gru_bih = np.asarray(inputs["gru_bih"], f32)
    gru_bhh = np.asarray(inputs["gru_bhh"], f32)
    lstm_wih = np.asarray(inputs["lstm_wih"], f32)
    lstm_whh = np.asarray(inputs["lstm_whh"], f32)
    lstm_bih = np.asarray(inputs["lstm_bih"], f32)
    lstm_bhh = np.asarray(inputs["lstm_bhh"], f32)
    out_w = np.asarray(inputs["out_w"], f32)
    out_b = np.asarray(inputs["out_b"], f32)
    fc_z_w = np.asarray(inputs["fc_z_w"], f32)
    fc_z_b = np.asarray(inputs["fc_z_b"], f32)

    xg_tab = emb @ gru_wih.T + gru_bih
    xg_tab[:, 0:H] += gru_bhh[0:H]
    xg_tab[:, H : 2 * H] += gru_bhh[H : 2 * H]

    hl_init = np.zeros((E + 1, BC), f32)
    hl_init[E, :] = 1.0

    wih_lT = np.ascontiguousarray(lstm_wih.T).astype(f32)
    whh_laug = np.concatenate(
        [lstm_whh.T, (lstm_bih + lstm_bhh)[None, :]], axis=0).astype(f32)
    out_waug = np.concatenate([out_w.T, out_b[None, :]], axis=0).astype(f32)

    c = np.ascontiguousarray
    return {
        "hl_init": hl_init,
        "xg_tab": c(xg_tab.astype(f32)),
        "bhh_n": c(gru_bhh[2 * H : 3 * H][:, None]),
        "whh_T": c(gru_whh.T),
        "fcw_T": c(fc_z_w.T),
        "fc_b": c(fc_z_b[:, None]),
        "wih_lT": c(wih_lT),
        "whh_laug": c(whh_laug),
        "out_waug": c(out_waug),
    }


_NC_CACHE = {}


def _build(num_devices=N_CORES, reps=1):
    key = (num_devices, reps)
    if key in _NC_CACHE:
        return _NC_CACHE[key]
    nc = bacc.Bacc("TRN2", target_bir_lowering=False, debug=False,
                   num_devices=num_devices)
    d = {}
    for name, shape, dt_ in [
        ("zT", [L, BC], F32R), ("O", [T, V, BC], F32R),
        ("xg_tab", [V, 3 * H], F32R), ("bhh_n", [H, 1], F32),
        ("whh_T", [H, 3 * H], F32R),
        ("fcw_T", [L, H], F32R), ("fc_b", [H, 1], F32),
        ("wih_lT", [H, 4 * E], F32R), ("whh_laug", [E + 1, 4 * E], F32R),
        ("out_waug", [E + 1, V], F32R), ("hl_init", [E + 1, BC], F32R),
    ]:
        d[name] = nc.dram_tensor(name, shape, dt_, kind="ExternalInput").ap()
    d["logits"] = nc.dram_tensor("logits", [T, V, BC], F32,
                                 kind="ExternalOutput").ap()
    with tile.TileContext(nc) as tc:
        with ExitStack() as ctx:
            _emit(nc, tc, d, ctx, reps=reps)
    nc.compile()
    _NC_CACHE[key] = nc
    return nc


def build_in_maps(inputs):
    prep = _host_prep(inputs)
    z = np.asarray(inputs["z"], np.float32)
    x_in = np.asarray(inputs["x_in"])
    zT = np.ascontiguousarray(z.T)                       # (L, B)
    # one-hot [T, V, B]
    O = (x_in[:, :, None] == np.arange(V)[None, None, :]).astype(np.float32)
    O = np.ascontiguousarray(np.transpose(O, (1, 2, 0)))  # (T, V, B)
    in_maps = []
    for ci in range(N_CORES):
        bs = slice(ci * BC, (ci + 1) * BC)
        m = dict(prep)
        m["zT"] = np.ascontiguousarray(zT[:, bs])
        m["O"] = np.ascontiguousarray(O[:, :, bs])
        in_maps.append(m)
    return in_maps


def assemble_output(results):
    outs = []
    for ci in range(N_CORES):
        lg = results[ci]["logits"]                       # (T, V, BC)
        outs.append(np.ascontiguousarray(np.transpose(lg, (2, 0, 1))))
    return np.concatenate(outs, axis=0).astype(np.float32)  # (B, T, V)


def kernel(**inputs) -> np.ndarray:
    nc = _build()
    in_maps = build_in_maps(inputs)
    res = run_bass_kernel_spmd(nc, in_maps, list(range(N_CORES)))
    return assemble_output(res.results)

